# revision 6
# baseline (speedup 1.0000x reference)
"""Trainium2 Bass kernel for nn_CXINGeneral_1425929142863 (GNN message passing).

Math (per branch b, epsilon=0):
    agg_b = A_b @ x_src_b               (gather + segment-sum in IN_CH=128 space)
    h_b   = relu-MLP_b( agg_b @ W_b + x_target )   (3 layers)
    out   = concat(h0, h1) @ Wm + bm

v2 design (vs v1 per-chunk indirect-DMA + fp32 matmuls):
  - all matmuls bf16 (PSUM accumulates fp32); host pre-converts tables/weights
  - gathers via batched SWDGE dma_gather (mlp library): one instruction per
    ~7-block group instead of one 1.1us indirect DMA per 128-edge chunk.
    int16 index limit (<32768) handled by per-(core, branch, row-half)
    compacted source tables (~22K unique cols each).
  - one-hot scatter matrices generated on device: per chunk one DVE
    tensor_scalar (iota is_equal d) * val -> S bf16. No 58MB/core S stream.
  - head matmul folded into MLP layer 1 on host: agg @ (W @ mW0) +
    (x_target @ mW0 + b0); the x-term enters PSUM via an identity matmul.
  - bias+relu on the Activation engine (per-partition bias, transposed
    activations [ch, rows]); merge bias via a rank-1 ones x bm matmul.
  - scatter(blocks of 128 rows, k chunks/block) and dense windows (1024 rows)
    interleaved so gather DMA, DVE S-gen, PE matmuls and Act relus overlap.

Distribution: target rows sharded 8 ways (6250/core); edge lists partitioned
host-side by target ownership; weights replicated; no collectives.
"""

import sys
import types

import numpy as np
import ml_dtypes

import concourse.bass as bass
import concourse.mybir as mybir
import concourse.tile as tile
from concourse import bacc
from concourse import library_config
import concourse.bass_utils as bass_utils
from concourse.bass_utils import run_bass_kernel_spmd

F32 = mybir.dt.float32
BF16 = mybir.dt.bfloat16
I32 = mybir.dt.int32
I16 = mybir.dt.int16
BF = ml_dtypes.bfloat16


def _install_profile_hook():
    """This container's antenv lacks axon_hooks; reconstruct so trace=True works."""
    try:
        import antenv.axon_hooks  # noqa: F401
        return
    except ImportError:
        pass
    try:
        from trn_agent_boot.trn_boot import _ntff_profile_via_ctypes
    except ImportError:
        return
    mod = types.ModuleType("antenv.axon_hooks")
    hook = _ntff_profile_via_ctypes("/opt/axon/libaxon_pjrt.so")
    mod.get_axon_ntff_profile_hook = lambda: hook
    sys.modules["antenv.axon_hooks"] = mod
    bass_utils.upload_artifacts = lambda tmpdir: f"local:{tmpdir}"


class Cfg:
    def __init__(self):
        self.N_T = 50000
        self.N_S = 100000
        self.E = 400000
        self.NC = 8
        self.IN_CH = 128
        self.OUT_CH = 256
        self.N_MLP = 3
        self.NT_LOC = self.N_T // self.NC      # 6250
        self.R = 128                            # rows per scatter block
        self.NBLK = -(-self.NT_LOC // self.R)   # 49
        self.HALF_BLK = 25                      # blocks 0-24 | 25-48
        self.GROUPS = [7, 7, 7, 4, 7, 7, 7, 3]  # blocks per gather/psum group
        self.WIN = 1024                         # dense row-window width


CFG = Cfg()


# ----------------------------------------------------------------- host prep

def _prep_edges(cfg, rows, cols, vals):
    """Partition + sort one branch's edges; emit per-core gather/S streams.

    Chunks: per (core, block) exactly k slots of 128 edges (k = global max).
    Within a block edges are ordered by col (gather locality). Gather tables
    are compacted per (core, row-half) so local indices fit int16.

    Returns dict with per-core arrays + k.
    """
    rows = np.asarray(rows, np.int64)
    cols = np.asarray(cols, np.int64)
    vals = np.asarray(vals, np.float32)

    core = rows // cfg.NT_LOC
    lrow = rows % cfg.NT_LOC
    blk = lrow // cfg.R
    d = lrow % cfg.R

    group = core * cfg.NBLK + blk
    order = np.lexsort((cols, group))
    g_s = group[order]
    core_s = core[order]
    blk_s = blk[order]
    d_s = d[order].astype(np.float32)
    cols_s = cols[order]
    vals_s = vals[order]

    n_groups = cfg.NC * cfg.NBLK
    counts = np.bincount(g_s, minlength=n_groups)
    k = int((counts.max() + 127) // 128)
    C = cfg.NBLK * k

    starts = np.zeros(n_groups, np.int64)
    np.cumsum(counts[:-1], out=starts[1:])
    rank = np.arange(len(rows)) - starts[g_s]

    chunk = blk_s * k + rank // 128
    lane = rank % 128

    d_arr = np.zeros((cfg.NC, 128, C), np.float32)
    v_arr = np.zeros((cfg.NC, 128, C), np.float32)
    gcol = np.zeros((cfg.NC, 128, C), np.int64)   # global col per slot (pad=first col of half)
    d_arr[core_s, lane, chunk] = d_s
    v_arr[core_s, lane, chunk] = vals_s
    gcol[core_s, lane, chunk] = cols_s

    return dict(d=d_arr, v=v_arr, gcol=gcol, k=k)


def _build_tables_and_idx(cfg, gcol, k, xsrc_bf):
    """Per-(core, half) compacted tables + int16 index stream.

    gcol: [NC, 128, C] global col ids (pads = 0).
    Returns tabs[core][half] (np bf16 [u,128]), idx16[core] ([128, C*8] i16).
    """
    C = cfg.NBLK * k
    half_chunks = [cfg.HALF_BLK * k, (cfg.NBLK - cfg.HALF_BLK) * k]
    tabs = []
    idxs = []
    for c in range(cfg.NC):
        tab_h = []
        idx_flat = np.zeros(C * 128, np.int16)
        for h in (0, 1):
            c0 = 0 if h == 0 else half_chunks[0]
            c1 = half_chunks[0] if h == 0 else C
            g = gcol[c, :, c0:c1]                     # [128, chunks_h]
            uniq, inv = np.unique(g, return_inverse=True)
            assert len(uniq) < 32768, f"half table too big: {len(uniq)}"
            inv = inv.reshape(g.shape).astype(np.int16)
            tab_h.append(np.ascontiguousarray(xsrc_bf[uniq]))
            # idx j = chunk*128 + lane -> inv[lane, chunk-c0]
            idx_flat[c0 * 128:c1 * 128] = inv.T.reshape(-1)
        tabs.append(tab_h)
        # wrap 16, replicate 8x across partitions
        idx16 = np.tile(idx_flat.reshape(-1, 16).T, (8, 1))
        idxs.append(np.ascontiguousarray(idx16))
    return tabs, idxs


def prep_inputs(cfg, inputs):
    x_target = np.asarray(inputs["x_target"], np.float32)
    xs_bf = [np.asarray(inputs[f"x_src{b}"], np.float32).astype(BF) for b in (0, 1)]

    eb = [
        _prep_edges(cfg, inputs["rows0"], inputs["cols0"], inputs["vals0"]),
        _prep_edges(cfg, inputs["rows1"], inputs["cols1"], inputs["vals1"]),
    ]
    k = (eb[0]["k"], eb[1]["k"])
    tabs, idxs = zip(*[
        _build_tables_and_idx(cfg, eb[b]["gcol"], k[b], xs_bf[b]) for b in (0, 1)
    ])  # tabs[b][core][half], idxs[b][core]
    # pad half-tables to a shared max size per (branch, half) so the SPMD
    # program has fixed shapes
    tab_sizes = [[max(tabs[b][c][h].shape[0] for c in range(cfg.NC)) for h in (0, 1)]
                 for b in (0, 1)]

    mW = [np.asarray(inputs[f"mlp_W{b}"], np.float32) for b in (0, 1)]
    mB = [np.asarray(inputs[f"mlp_b{b}"], np.float32) for b in (0, 1)]
    W = [np.asarray(inputs[f"W{b}"], np.float32) for b in (0, 1)]

    # fused layer 1:  h1 = relu( agg @ (W @ mW0)  +  (x_target @ mW0 + b0) )
    w0p = []       # [128, 2*128] bf16, ocb-major
    xt1 = []       # [OUT, N_T] bf16 transposed
    for b in (0, 1):
        fw = (W[b] @ mW[b][0]).astype(np.float32)          # [128, 256]
        w0p.append(np.ascontiguousarray(fw.astype(BF)))
        ft = (x_target @ mW[b][0] + mB[b][0]).astype(np.float32)  # [N_T, 256]
        xt1.append(np.ascontiguousarray(ft.T.astype(BF)))  # [256, N_T]

    # layers 2,3 weights packed [128, (l,icb,ocb)*128] bf16
    mlpw = []
    bvec = []
    for b in (0, 1):
        blocks = []
        for l in (1, 2):
            for icb in range(2):
                for ocb in range(2):
                    blocks.append(mW[b][l][icb * 128:(icb + 1) * 128,
                                           ocb * 128:(ocb + 1) * 128])
        mlpw.append(np.ascontiguousarray(np.concatenate(blocks, axis=1).astype(BF)))
        cols_ = []
        for l in (1, 2):
            for ocb in range(2):
                cols_.append(mB[b][l][ocb * 128:(ocb + 1) * 128][:, None])
        bvec.append(np.ascontiguousarray(np.concatenate(cols_, axis=1)))  # [128,4] f32

    Wm = np.asarray(inputs["Wm"], np.float32)   # [512, 256]
    wm = np.ascontiguousarray(np.concatenate(
        [Wm[t * 128:(t + 1) * 128, :] for t in range(4)], axis=1).astype(BF))
    bm = np.ascontiguousarray(np.asarray(inputs["bm"], np.float32)[None, :].astype(BF))
    ones = np.ones((1, 128), np.float32).astype(BF)
    iota = np.tile(np.arange(128, dtype=np.float32), (128, 1)).astype(BF)
    ident = np.eye(128, dtype=np.float32).astype(BF)

    in_maps = []
    for c in range(cfg.NC):
        m = {
            "w0p0": w0p[0], "w0p1": w0p[1],
            "mlpw0": mlpw[0], "mlpw1": mlpw[1],
            "bv0": bvec[0], "bv1": bvec[1],
            "wm": wm, "bm": bm, "ones": ones, "iota": iota, "ident": ident,
        }
        for b in (0, 1):
            for h in (0, 1):
                t = tabs[b][c][h]
                tsz = tab_sizes[b][h]
                if t.shape[0] < tsz:
                    t = np.concatenate(
                        [t, np.zeros((tsz - t.shape[0], 128), BF)], axis=0)
                m[f"tab{b}{h}"] = np.ascontiguousarray(t)
            m[f"idx{b}"] = idxs[b][c]
            m[f"dd{b}"] = np.ascontiguousarray(eb[b]["d"][c])
            m[f"vv{b}"] = np.ascontiguousarray(eb[b]["v"][c])
            m[f"xt1{b}"] = np.ascontiguousarray(
                xt1[b][:, c * cfg.NT_LOC:(c + 1) * cfg.NT_LOC])
        in_maps.append(m)
    return in_maps, k, tab_sizes


# ------------------------------------------------------------------- builder

def build(cfg, k, tab_sizes):
    nc = bacc.Bacc("TRN2", target_bir_lowering=False, debug=False)

    C = [cfg.NBLK * k[0], cfg.NBLK * k[1]]
    tab_d = [[nc.declare_dram_parameter(f"tab{b}{h}", [tab_sizes[b][h], 128], BF16,
                                        isOutput=False) for h in (0, 1)] for b in (0, 1)]
    idx_d = [nc.declare_dram_parameter(f"idx{b}", [128, C[b] * 8], I16, isOutput=False)
             for b in (0, 1)]
    dd_d = [nc.declare_dram_parameter(f"dd{b}", [128, C[b]], F32, isOutput=False)
            for b in (0, 1)]
    vv_d = [nc.declare_dram_parameter(f"vv{b}", [128, C[b]], F32, isOutput=False)
            for b in (0, 1)]
    xt1_d = [nc.declare_dram_parameter(f"xt1{b}", [cfg.OUT_CH, cfg.NT_LOC], BF16,
                                       isOutput=False) for b in (0, 1)]
    w0p_d = [nc.declare_dram_parameter(f"w0p{b}", [128, 256], BF16, isOutput=False)
             for b in (0, 1)]
    mlpw_d = [nc.declare_dram_parameter(f"mlpw{b}", [128, 8 * 128], BF16,
                                        isOutput=False) for b in (0, 1)]
    bv_d = [nc.declare_dram_parameter(f"bv{b}", [128, 4], F32, isOutput=False)
            for b in (0, 1)]
    wm_d = nc.declare_dram_parameter("wm", [128, 4 * 256], BF16, isOutput=False)
    bm_d = nc.declare_dram_parameter("bm", [1, 256], BF16, isOutput=False)
    ones_d = nc.declare_dram_parameter("ones", [1, 128], BF16, isOutput=False)
    iota_d = nc.declare_dram_parameter("iota", [128, 128], BF16, isOutput=False)
    ident_d = nc.declare_dram_parameter("ident", [128, 128], BF16, isOutput=False)
    out_d = nc.declare_dram_parameter("out", [cfg.NT_LOC, cfg.OUT_CH], F32,
                                      isOutput=True)

    groups = cfg.GROUPS
    gstart = np.cumsum([0] + groups).tolist()   # block index where group g starts
    max_gblk = max(groups)

    # dense windows
    wins = []
    w0 = 0
    while w0 < cfg.NT_LOC:
        wins.append((w0, min(cfg.WIN, cfg.NT_LOC - w0)))
        w0 += cfg.WIN
    # group after which window w is fully covered (its last block copied):
    # GROUPS=[7,7,7,4,7,7,7,3] -> cum=[7,14,21,25,32,39,46,49]; window w needs
    # block 8w+7 (block 48 for the tail window)
    cum = np.cumsum(groups).tolist()
    win_after_group = {}
    for w, (w0, wl) in enumerate(
            [(i * cfg.WIN, min(cfg.WIN, cfg.NT_LOC - i * cfg.WIN))
             for i in range(-(-cfg.NT_LOC // cfg.WIN))]):
        need = (w0 + wl - 1) // cfg.R
        g_ready = next(g for g in range(len(groups)) if cum[g] - 1 >= need)
        win_after_group.setdefault(g_ready, []).append(w)

    with tile.TileContext(nc) as tc:
        with (
            tc.tile_pool(name="wpool", bufs=1) as wpool,
            tc.tile_pool(name="hbig", bufs=1) as hbig,
            tc.tile_pool(name="gat", bufs=2) as gat,
            tc.tile_pool(name="spool", bufs=8) as spool,
            tc.tile_pool(name="xtp", bufs=3) as xtp,
            tc.tile_pool(name="hwin", bufs=2) as hwin,
            tc.tile_pool(name="outp", bufs=4) as outp,
            tc.tile_pool(name="pscat", bufs=2, space="PSUM") as pscat,
            tc.tile_pool(name="pdense", bufs=2, space="PSUM") as pdense,
        ):
            nc.gpsimd.load_library(library_config.mlp)

            # --- resident small tensors
            def preload(name, shape, dt, src):
                t = wpool.tile(shape, dt, tag=name)
                nc.sync.dma_start(out=t[:], in_=src[:])
                return t

            idx_sb = [preload(f"idx{b}", [128, C[b] * 8], I16, idx_d[b]) for b in (0, 1)]
            dd_sb = [preload(f"dd{b}", [128, C[b]], F32, dd_d[b]) for b in (0, 1)]
            vv_sb = [preload(f"vv{b}", [128, C[b]], F32, vv_d[b]) for b in (0, 1)]
            w0p_sb = [preload(f"w0p{b}", [128, 256], BF16, w0p_d[b]) for b in (0, 1)]
            mlpw_sb = [preload(f"mlpw{b}", [128, 8 * 128], BF16, mlpw_d[b])
                       for b in (0, 1)]
            bv_sb = [preload(f"bv{b}", [128, 4], F32, bv_d[b]) for b in (0, 1)]
            wm_sb = preload("wm", [128, 4 * 256], BF16, wm_d)
            bm_sb = preload("bm", [1, 256], BF16, bm_d)
            ones_sb = preload("ones", [1, 128], BF16, ones_d)
            iota_sb = preload("iota", [128, 128], BF16, iota_d)
            ident_sb = preload("ident", [128, 128], BF16, ident_d)

            # --- persistent activations (bf16, transposed [ch, rows])
            aggT = [hbig.tile([128, cfg.NBLK * cfg.R], BF16, tag=f"agg{b}",
                              name=f"agg{b}") for b in (0, 1)]
            hT = [[hbig.tile([128, cfg.NT_LOC], BF16, tag=f"h{b}{hf}",
                             name=f"h{b}{hf}") for hf in (0, 1)] for b in (0, 1)]

            def dense_window(b, w0, wl):
                kb = k[b]  # noqa: F841
                cur = []
                for ocb in range(2):
                    pd = pdense.tile([128, cfg.WIN], F32, tag="pd")
                    xw = xtp.tile([128, cfg.WIN], BF16, tag="xt")
                    nc.scalar.dma_start(
                        out=xw[:, :wl],
                        in_=xt1_d[b][ocb * 128:(ocb + 1) * 128, w0:w0 + wl])
                    for half in range(2):
                        h0 = half * 512
                        hl = min(512, wl - h0)
                        if hl <= 0:
                            continue
                        nc.tensor.matmul(
                            out=pd[:, h0:h0 + hl],
                            lhsT=w0p_sb[b][:, ocb * 128:(ocb + 1) * 128],
                            rhs=aggT[b][:, w0 + h0:w0 + h0 + hl],
                            start=True, stop=False)
                        nc.tensor.matmul(
                            out=pd[:, h0:h0 + hl],
                            lhsT=ident_sb[:],
                            rhs=xw[:, h0:h0 + hl],
                            start=False, stop=True)
                    h1 = hwin.tile([128, cfg.WIN], BF16, tag=f"h1{ocb}")
                    nc.scalar.activation(
                        out=h1[:, :wl], in_=pd[:, :wl],
                        func=mybir.ActivationFunctionType.Relu,
                        bias=0.0, scale=1.0)
                    cur.append(h1)
                for l in range(2):
                    nxt = []
                    for ocb in range(2):
                        pm = pdense.tile([128, cfg.WIN], F32, tag="pd")
                        for half in range(2):
                            h0 = half * 512
                            hl = min(512, wl - h0)
                            if hl <= 0:
                                continue
                            for icb in range(2):
                                nc.tensor.matmul(
                                    out=pm[:, h0:h0 + hl],
                                    lhsT=mlpw_sb[b][:, ((l * 2 + icb) * 2 + ocb) * 128:
                                                    ((l * 2 + icb) * 2 + ocb) * 128 + 128],
                                    rhs=cur[icb][:, h0:h0 + hl],
                                    start=(icb == 0), stop=(icb == 1))
                        if l == 1:
                            out_ap = hT[b][ocb][:, w0:w0 + wl]
                        else:
                            hn = hwin.tile([128, cfg.WIN], BF16, tag=f"h2{ocb}")
                            out_ap = hn[:, :wl]
                        nc.scalar.activation(
                            out=out_ap, in_=pm[:, :wl],
                            func=mybir.ActivationFunctionType.Relu,
                            bias=bv_sb[b][:, l * 2 + ocb:l * 2 + ocb + 1],
                            scale=1.0)
                        if l == 0:
                            nxt.append(hn)
                    cur = nxt

            # --- interleaved scatter + dense
            for g, nblk in enumerate(groups):
                b0 = gstart[g]
                half = 0 if b0 < cfg.HALF_BLK else 1
                for b in (0, 1):
                    kb = k[b]
                    nch = nblk * kb
                    c0 = b0 * kb
                    gx = gat.tile([128, max_gblk * max(k) * 128], BF16, tag="gx")
                    nc.gpsimd.dma_gather(
                        out_ap=gx[:, :nch * 128].rearrange("p (s c) -> p s c", s=nch),
                        in_ap=tab_d[b][half][:],
                        idxs_ap=idx_sb[b][:, c0 * 8:(c0 + nch) * 8],
                        num_idxs=nch * 128,
                        num_idxs_reg=nch * 128,
                        elem_size=128,
                        single_packet=False,
                    )
                    psc = pscat.tile([128, max_gblk * cfg.R], F32, tag="ps")
                    for bi in range(nblk):
                        for j in range(kb):
                            c = c0 + bi * kb + j
                            s_t = spool.tile([128, cfg.R], BF16, tag="s")
                            nc.vector.tensor_scalar(
                                out=s_t[:], in0=iota_sb[:],
                                scalar1=dd_sb[b][:, c:c + 1],
                                scalar2=vv_sb[b][:, c:c + 1],
                                op0=mybir.AluOpType.is_equal,
                                op1=mybir.AluOpType.mult)
                            nc.tensor.matmul(
                                out=psc[:, bi * cfg.R:(bi + 1) * cfg.R],
                                lhsT=gx[:, (bi * kb + j) * 128:(bi * kb + j) * 128 + 128],
                                rhs=s_t[:],
                                start=(j == 0), stop=(j == kb - 1))
                    nc.vector.tensor_copy(
                        out=aggT[b][:, b0 * cfg.R:(b0 + nblk) * cfg.R],
                        in_=psc[:, :nblk * cfg.R])
                for w in win_after_group.get(g, []):
                    for b in (0, 1):
                        dense_window(b, wins[w][0], wins[w][1])

            # --- merge
            nrt = -(-cfg.NT_LOC // 128)
            for t in range(nrt):
                r0 = t * 128
                rl = min(128, cfg.NT_LOC - r0)
                pm = pdense.tile([128, cfg.WIN], F32, tag="pd")
                for ic in range(4):
                    nc.tensor.matmul(
                        out=pm[:rl, :cfg.OUT_CH],
                        lhsT=hT[ic // 2][ic % 2][:, r0:r0 + rl],
                        rhs=wm_sb[:, ic * cfg.OUT_CH:(ic + 1) * cfg.OUT_CH],
                        start=(ic == 0), stop=False)
                nc.tensor.matmul(
                    out=pm[:rl, :cfg.OUT_CH],
                    lhsT=ones_sb[:, :rl],
                    rhs=bm_sb[:],
                    start=False, stop=True)
                o_sb = outp.tile([128, cfg.OUT_CH], F32, tag="o")
                nc.scalar.copy(out=o_sb[:rl], in_=pm[:rl, :cfg.OUT_CH])
                nc.sync.dma_start(out=out_d[r0:r0 + rl, :], in_=o_sb[:rl])

    nc.compile()
    return nc


# -------------------------------------------------------------------- runner

_CACHE = {}


def kernel(**inputs) -> np.ndarray:
    _install_profile_hook()
    cfg = CFG
    in_maps, k, tab_sizes = prep_inputs(cfg, inputs)
    key = ("v2", k, tuple(map(tuple, tab_sizes)))
    if key not in _CACHE:
        _CACHE[key] = build(cfg, k, tab_sizes)
    nc = _CACHE[key]
    trace = bool(int(__import__("os").environ.get("KERNEL_TRACE", "0")))
    r = run_bass_kernel_spmd(nc, in_maps, core_ids=list(range(cfg.NC)), trace=trace)
    kernel.last_result = r
    out = np.concatenate([r.results[c]["out"] for c in range(cfg.NC)], axis=0)
    return out.astype(np.float32)


kernel.last_result = None
